# revision 18
# baseline (speedup 1.0000x reference)
"""Trainium2 Bass kernel for the EABlock problem.

Math (per batch sample, x: [c=256, n=16384]):
    y    = conv1_w @ x + conv1_b                      (1x1 conv)
    attn = softmax_n(mk_w @ y)                        (softmax over n)
    attn = attn / (1e-9 + attn.sum(d))                (column-normalize over d=64)
    z    = conv2_w @ (mv_w @ attn)
    out  = relu(bn(z) + x)

Folds used:
  * conv1_b shifts each d-row of the logits by a constant -> cancels in
    row-softmax -> dropped entirely.
  * A = mk_w @ conv1_w          [64, 256]  (logits = A @ x)
  * B = diag(bn_inv) @ conv2_w @ mv_w   [256, 64]
    out = relu(B @ attn_norm + bn_bias + x)
  * softmax without max-subtraction (logits ~ N(0, 16), exp is safe in
    fp32) -> E = exp(logits), row-sums S via the ACT accum_out port.
  * column-normalizer and 1/S folded as:
      z[:, j] = sum_d B[:, d] * invS[d] * E[d, j] / (1e-9 + s[j])
    with s[j] = sum_d invS[d] E[d, j] computed by one PE matmul against a
    block-diagonal invS matrix, broadcast to 128 partitions for free.

Layout per core (2 samples, processed one after the other):
  x sample  = 4 logical tiles [128, 8192] (c-half x n-half), stored as
              8 SBUF tiles [128, 4096]; n-halves are STACKED on the
      partition axis for everything attn-sized: rows 0-63 = d of n-half 0,
      rows 64-127 = d of n-half 1, so DVE/ACT passes run at full width.
  E [128, 8192] float32r stays resident; phase 2 re-reads x from SBUF for
  the residual -> x is read from HBM exactly once (DMA-roofline design).

Matmul dtypes: logits in plain fp32 (exact, 4 cyc/row); the z / colsum
matmuls in float32r (TF32, 1 cyc/row) whose inputs are produced by
DVE/ACT ops anyway (the producer-rounding rule is satisfied for free).
"""
import os
import sys

sys.path.insert(0, "/opt/trn_rl_repo")

import numpy as np

import concourse.bacc as bacc
import concourse.tile as tile
from concourse import mybir
from concourse.bass_utils import run_bass_kernel_spmd

f32 = mybir.dt.float32
f32r = mybir.dt.float32r

B_FULL, C, H, W, D = 16, 256, 128, 128, 64
N = H * W                    # 16384 spatial positions
NCORES = 8
SPC = B_FULL // NCORES       # samples per core = 2
NH = N // 2                  # 8192, one n-half
XT = 2048                    # x sub-tile width
CHUNK = 512                  # phase chunk width (PSUM bank)
NCHUNK = NH // CHUNK         # 16 chunks per sample
BN_EPS = 1e-5

_cache = {}


def _build():
    nc = bacc.Bacc()
    x_d = nc.declare_dram_parameter("xin", [SPC, C, N], f32, isOutput=False)
    a1_d = nc.declare_dram_parameter("a1t", [128, D], f32, isOutput=False)
    a2_d = nc.declare_dram_parameter("a2t", [128, D], f32, isOutput=False)
    w0_d = nc.declare_dram_parameter("w0", [128, 128], f32, isOutput=False)
    w1_d = nc.declare_dram_parameter("w1", [128, 128], f32, isOutput=False)
    fo_d = nc.declare_dram_parameter("fold", [128, 128], f32, isOutput=False)
    mk_d = nc.declare_dram_parameter("mask", [128, 128], f32, isOutput=False)
    bb_d = nc.declare_dram_parameter("bnb", [128, 2], f32, isOutput=False)
    id_d = nc.declare_dram_parameter("ident", [128, 128], f32, isOutput=False)
    y_d = nc.declare_dram_parameter("y", [SPC, C, N], f32, isOutput=True)

    with tile.TileContext(nc) as tc:
        with (
            tc.tile_pool(name="consts", bufs=1) as cpool,
            tc.tile_pool(name="xp", bufs=8) as xp,
            tc.tile_pool(name="ep", bufs=2) as ep,
            tc.tile_pool(name="small", bufs=2) as sp,
            tc.tile_pool(name="stage", bufs=2) as stp,
            tc.tile_pool(name="aps", bufs=2, space="PSUM") as aps,
            tc.tile_pool(name="sps", bufs=2, space="PSUM") as sps_pool,
            tc.tile_pool(name="zps", bufs=2, space="PSUM") as zps_pool,
        ):
            a1t = cpool.tile([128, D], f32)
            a2t = cpool.tile([128, D], f32)
            w0 = cpool.tile([128, 128], f32)
            w1 = cpool.tile([128, 128], f32)
            fold = cpool.tile([128, 128], f32)
            mask = cpool.tile([128, 128], f32)
            bnb = cpool.tile([128, 2], f32)
            ident = cpool.tile([128, 128], f32)
            nc.sync.dma_start(out=a1t, in_=a1_d[:, :])
            nc.sync.dma_start(out=a2t, in_=a2_d[:, :])
            nc.sync.dma_start(out=w0, in_=w0_d[:, :])
            nc.sync.dma_start(out=w1, in_=w1_d[:, :])
            nc.sync.dma_start(out=fold, in_=fo_d[:, :])
            nc.sync.dma_start(out=mask, in_=mk_d[:, :])
            nc.sync.dma_start(out=bnb, in_=bb_d[:, :])
            nc.sync.dma_start(out=ident, in_=id_d[:, :])
            ws = [w0, w1]
            NQ = NH // XT

            for s in range(SPC):
                # x tiles [128, 2, XT]: (c-half h, quarter q); dim 1 is the
                # n-half, so both n-halves of a chunk live in one tile and
                # the z tail handles them in single [128, 2*CHUNK] ops.
                xt = {}
                for q in range(NQ):
                    for h in range(2):
                        t = xp.tile([128, 2, XT], f32, tag="x",
                                    name=f"x_s{s}_h{h}_q{q}")
                        src_ap = (x_d[s, 128 * h:128 * h + 128, :]
                                  .rearrange("p (a b) -> p a b", a=2)
                                  [:, :, q * XT:q * XT + XT])
                        nc.sync.dma_start(out=t, in_=src_ap)
                        xt[h, q] = t

                # ---- phase 1: logits -> exp -> row-sums ----
                E = ep.tile([128, NH], f32r, tag="E", name=f"E_s{s}")
                s_cols = sp.tile([128, NCHUNK], f32, tag="scols",
                                 name=f"scols_s{s}")
                for j in range(NCHUNK):
                    q, off = divmod(j * CHUNK, XT)
                    ps = aps.tile([128, CHUNK], f32, tag="attn")
                    for nh in range(2):
                        pr = ps[64 * nh:64 * nh + 64, :]
                        nc.tensor.matmul(
                            pr, lhsT=a1t,
                            rhs=xt[0, q][:, nh, off:off + CHUNK],
                            start=True, stop=False)
                        nc.tensor.matmul(
                            pr, lhsT=a2t,
                            rhs=xt[1, q][:, nh, off:off + CHUNK],
                            start=False, stop=True)
                    nc.scalar.activation(
                        out=E[:, j * CHUNK:(j + 1) * CHUNK], in_=ps,
                        func=mybir.ActivationFunctionType.Exp,
                        bias=0.0, scale=1.0,
                        accum_out=s_cols[:, j:j + 1])

                # ---- normalizers ----
                s_half = sp.tile([128, 1], f32, tag="shalf", name=f"shalf_s{s}")
                nc.vector.reduce_sum(out=s_half, in_=s_cols,
                                     axis=mybir.AxisListType.X)
                fps = sps_pool.tile([128, 1], f32, tag="sb")
                nc.tensor.matmul(fps, lhsT=fold, rhs=s_half,
                                 start=True, stop=True)
                invs = sp.tile([128, 1], f32, tag="invs", name=f"invs_s{s}")
                nc.vector.reciprocal(out=invs, in_=fps)
                blk = sp.tile([128, 128], f32r, tag="blk", name=f"blk_s{s}")
                nc.vector.tensor_scalar_mul(blk, in0=mask, scalar1=invs)
                wsc = []
                for h in range(2):
                    wt = sp.tile([128, 128], f32r, tag=f"wsc{h}",
                                 name=f"wsc{h}_s{s}")
                    nc.vector.tensor_scalar_mul(wt, in0=ws[h], scalar1=invs)
                    wsc.append(wt)

                # ---- phase 2 ----
                for j in range(NCHUNK):
                    q, off = divmod(j * CHUNK, XT)
                    Ej = E[:, j * CHUNK:(j + 1) * CHUNK]
                    sps = sps_pool.tile([128, CHUNK], f32, tag="sb")
                    nc.tensor.matmul(sps, lhsT=blk, rhs=Ej,
                                     start=True, stop=True)
                    # reciprocal in place in PSUM. The reference's +1e-9
                    # is dropped: data-measured min column-sum is 7e-6, so
                    # the eps shifts the result by <1.5e-4 of the (small)
                    # attention branch -- far below fp32-gate noise.
                    nc.vector.reciprocal_approx_fast(out=sps, in_=sps)
                    nc.vector.tensor_mul(Ej, Ej.bitcast(f32), sps)
                    if True:
                        for h in range(2):
                            # z_big [128, 2, CHUNK]: both n-halves
                            zb = zps_pool.tile([128, 2, CHUNK], f32, tag="z")
                            for nh in range(2):
                                nc.tensor.matmul(
                                    zb[:, nh, :],
                                    lhsT=wsc[h][64 * nh:64 * nh + 64, :],
                                    rhs=Ej[64 * nh:64 * nh + 64, :],
                                    start=True, stop=True)
                            nc.vector.tensor_add(
                                zb, zb, xt[h, q][:, :, off:off + CHUNK])
                            st = stp.tile([128, 2, CHUNK], f32, tag="st")
                            nc.scalar.activation(
                                out=st, in_=zb,
                                func=mybir.ActivationFunctionType.Relu,
                                bias=bnb[:, h:h + 1], scale=1.0)
                            nc.sync.dma_start(
                                out=(y_d[s, 128 * h:128 * h + 128, :]
                                     .rearrange("p (a b) -> p a b", a=2)
                                     [:, :, j * CHUNK:(j + 1) * CHUNK]),
                                in_=st)
    nc.compile()
    return nc


def _consts(conv1_w, conv1_b, mk_w, mv_w, conv2_w, bn_gamma, bn_beta,
            bn_mean, bn_var):
    c1 = np.asarray(conv1_w, dtype=np.float64)
    mk = np.asarray(mk_w, dtype=np.float64)
    mv = np.asarray(mv_w, dtype=np.float64)
    c2 = np.asarray(conv2_w, dtype=np.float64)
    g = np.asarray(bn_gamma, dtype=np.float64)
    be = np.asarray(bn_beta, dtype=np.float64)
    mu = np.asarray(bn_mean, dtype=np.float64)
    va = np.asarray(bn_var, dtype=np.float64)

    A = mk @ c1                                    # [64, 256]
    inv = g / np.sqrt(va + BN_EPS)
    Bm = inv[:, None] * (c2 @ mv)                  # [256, 64]
    bias = be - mu * inv                           # [256]

    AT = np.ascontiguousarray(A.T, dtype=np.float32)      # [256, 64]
    a1t = AT[:128]
    a2t = AT[128:]
    wt = []
    for h in range(2):
        bh = np.ascontiguousarray(Bm[128 * h:128 * h + 128].T,
                                  dtype=np.float32)       # [64, 128]
        wt.append(np.concatenate([bh, bh], axis=0))       # [128, 128]
    k = np.arange(128)
    fold = (k[:, None] % 64 == k[None, :] % 64).astype(np.float32)
    mask = (k[:, None] // 64 == k[None, :] // 64).astype(np.float32)
    bnb = np.stack([bias[:128], bias[128:]], axis=1).astype(np.float32)
    ident = np.eye(128, dtype=np.float32)
    return {"a1t": a1t, "a2t": a2t, "w0": wt[0], "w1": wt[1],
            "fold": fold, "mask": mask, "bnb": bnb, "ident": ident}


def kernel(x, conv1_w, conv1_b, mk_w, mv_w, conv2_w, bn_gamma, bn_beta,
           bn_mean, bn_var):
    x = np.asarray(x, dtype=np.float32)
    consts = _consts(conv1_w, conv1_b, mk_w, mv_w, conv2_w, bn_gamma,
                     bn_beta, bn_mean, bn_var)
    if "nc" not in _cache:
        _cache["nc"] = _build()
    nc = _cache["nc"]

    xr = x.reshape(NCORES, SPC, C, N)
    in_maps = [dict(consts, xin=np.ascontiguousarray(xr[c]))
               for c in range(NCORES)]
    trace = bool(int(os.environ.get("KERNEL_TRACE", "0")))
    res = run_bass_kernel_spmd(nc, in_maps, list(range(NCORES)), trace=trace)
    _cache["exec_time_ns"] = res.exec_time_ns
    _cache["trace"] = res.instructions_and_trace
    out = np.stack([res.results[c]["y"] for c in range(NCORES)])
    return out.reshape(B_FULL, C, H, W)


# revision 19
# speedup vs baseline: 1.1472x; 1.1472x over previous
"""Trainium2 Bass kernel for the EABlock problem.

Math (per batch sample, x: [c=256, n=16384]):
    y    = conv1_w @ x + conv1_b                      (1x1 conv)
    attn = softmax_n(mk_w @ y)                        (softmax over n)
    attn = attn / (1e-9 + attn.sum(d))                (column-normalize over d=64)
    z    = conv2_w @ (mv_w @ attn)
    out  = relu(bn(z) + x)

Folds used:
  * conv1_b shifts each d-row of the logits by a constant -> cancels in
    row-softmax -> dropped entirely.
  * A = mk_w @ conv1_w          [64, 256]  (logits = A @ x)
  * B = diag(bn_inv) @ conv2_w @ mv_w   [256, 64]
    out = relu(B @ attn_norm + bn_bias + x)
  * softmax without max-subtraction (logits ~ N(0, 16), exp is safe in
    fp32) -> E = exp(logits), row-sums S via the ACT accum_out port.
  * column-normalizer and 1/S folded as:
      z[:, j] = sum_d B[:, d] * invS[d] * E[d, j] / (1e-9 + s[j])
    with s[j] = sum_d invS[d] E[d, j] computed by one PE matmul against a
    block-diagonal invS matrix, broadcast to 128 partitions for free.

Layout per core (2 samples, processed one after the other):
  x sample  = 4 logical tiles [128, 8192] (c-half x n-half), stored as
              8 SBUF tiles [128, 4096]; n-halves are STACKED on the
      partition axis for everything attn-sized: rows 0-63 = d of n-half 0,
      rows 64-127 = d of n-half 1, so DVE/ACT passes run at full width.
  E [128, 8192] float32r stays resident; phase 2 re-reads x from SBUF for
  the residual -> x is read from HBM exactly once (DMA-roofline design).

Matmul dtypes: logits in plain fp32 (exact, 4 cyc/row); the z / colsum
matmuls in float32r (TF32, 1 cyc/row) whose inputs are produced by
DVE/ACT ops anyway (the producer-rounding rule is satisfied for free).
"""
import os
import sys

sys.path.insert(0, "/opt/trn_rl_repo")

import numpy as np

import concourse.bacc as bacc
import concourse.tile as tile
from concourse import mybir
from concourse.bass_utils import run_bass_kernel_spmd

# If anything in the environment requests NTFF tracing (BASS_TRACE=1) but the
# image's antenv lacks the axon_hooks module, bass_utils would crash on
# import. Pre-seed a stub that reports "no hook" so tracing degrades to a
# plain run instead.
try:
    import antenv.axon_hooks  # noqa: F401
except ImportError:
    import types as _types

    _m = _types.ModuleType("antenv.axon_hooks")
    _m.get_axon_ntff_profile_hook = lambda: None
    _m.set_axon_ntff_profile_hook = lambda h: None
    sys.modules["antenv.axon_hooks"] = _m

f32 = mybir.dt.float32
f32r = mybir.dt.float32r

B_FULL, C, H, W, D = 16, 256, 128, 128, 64
N = H * W                    # 16384 spatial positions
NCORES = 8
SPC = B_FULL // NCORES       # samples per core = 2
NH = N // 2                  # 8192, one n-half
XT = 2048                    # x sub-tile width
CHUNK = 512                  # phase chunk width (PSUM bank)
NCHUNK = NH // CHUNK         # 16 chunks per sample
BN_EPS = 1e-5

_cache = {}


def _build():
    nc = bacc.Bacc()
    x_d = nc.declare_dram_parameter("xin", [SPC, C, N], f32, isOutput=False)
    a1_d = nc.declare_dram_parameter("a1t", [128, D], f32, isOutput=False)
    a2_d = nc.declare_dram_parameter("a2t", [128, D], f32, isOutput=False)
    w0_d = nc.declare_dram_parameter("w0", [128, 128], f32, isOutput=False)
    w1_d = nc.declare_dram_parameter("w1", [128, 128], f32, isOutput=False)
    fo_d = nc.declare_dram_parameter("fold", [128, 128], f32, isOutput=False)
    mk_d = nc.declare_dram_parameter("mask", [128, 128], f32, isOutput=False)
    bb_d = nc.declare_dram_parameter("bnb", [128, 2], f32, isOutput=False)
    id_d = nc.declare_dram_parameter("ident", [128, 128], f32, isOutput=False)
    y_d = nc.declare_dram_parameter("y", [SPC, C, N], f32, isOutput=True)

    with tile.TileContext(nc) as tc:
        with (
            tc.tile_pool(name="consts", bufs=1) as cpool,
            tc.tile_pool(name="xp", bufs=8) as xp,
            tc.tile_pool(name="ep", bufs=2) as ep,
            tc.tile_pool(name="small", bufs=2) as sp,
            tc.tile_pool(name="stage", bufs=2) as stp,
            tc.tile_pool(name="aps", bufs=2, space="PSUM") as aps,
            tc.tile_pool(name="sps", bufs=2, space="PSUM") as sps_pool,
            tc.tile_pool(name="zps", bufs=2, space="PSUM") as zps_pool,
        ):
            a1t = cpool.tile([128, D], f32)
            a2t = cpool.tile([128, D], f32)
            w0 = cpool.tile([128, 128], f32)
            w1 = cpool.tile([128, 128], f32)
            fold = cpool.tile([128, 128], f32)
            mask = cpool.tile([128, 128], f32)
            bnb = cpool.tile([128, 2], f32)
            ident = cpool.tile([128, 128], f32)
            nc.sync.dma_start(out=a1t, in_=a1_d[:, :])
            nc.sync.dma_start(out=a2t, in_=a2_d[:, :])
            nc.sync.dma_start(out=w0, in_=w0_d[:, :])
            nc.sync.dma_start(out=w1, in_=w1_d[:, :])
            nc.sync.dma_start(out=fold, in_=fo_d[:, :])
            nc.sync.dma_start(out=mask, in_=mk_d[:, :])
            nc.sync.dma_start(out=bnb, in_=bb_d[:, :])
            nc.sync.dma_start(out=ident, in_=id_d[:, :])
            ws = [w0, w1]
            NQ = NH // XT

            for s in range(SPC):
                # x tiles [128, 2, XT]: (c-half h, quarter q); dim 1 is the
                # n-half, so both n-halves of a chunk live in one tile and
                # the z tail handles them in single [128, 2*CHUNK] ops.
                xt = {}
                for q in range(NQ):
                    for h in range(2):
                        t = xp.tile([128, 2, XT], f32, tag="x",
                                    name=f"x_s{s}_h{h}_q{q}")
                        src_ap = (x_d[s, 128 * h:128 * h + 128, :]
                                  .rearrange("p (a b) -> p a b", a=2)
                                  [:, :, q * XT:q * XT + XT])
                        nc.sync.dma_start(out=t, in_=src_ap)
                        xt[h, q] = t

                # ---- phase 1: logits -> exp -> row-sums ----
                E = ep.tile([128, NH], f32r, tag="E", name=f"E_s{s}")
                s_cols = sp.tile([128, NCHUNK], f32, tag="scols",
                                 name=f"scols_s{s}")
                for j in range(NCHUNK):
                    q, off = divmod(j * CHUNK, XT)
                    ps = aps.tile([128, CHUNK], f32, tag="attn")
                    for nh in range(2):
                        pr = ps[64 * nh:64 * nh + 64, :]
                        nc.tensor.matmul(
                            pr, lhsT=a1t,
                            rhs=xt[0, q][:, nh, off:off + CHUNK],
                            start=True, stop=False)
                        nc.tensor.matmul(
                            pr, lhsT=a2t,
                            rhs=xt[1, q][:, nh, off:off + CHUNK],
                            start=False, stop=True)
                    nc.scalar.activation(
                        out=E[:, j * CHUNK:(j + 1) * CHUNK], in_=ps,
                        func=mybir.ActivationFunctionType.Exp,
                        bias=0.0, scale=1.0,
                        accum_out=s_cols[:, j:j + 1])

                # ---- normalizers ----
                s_half = sp.tile([128, 1], f32, tag="shalf", name=f"shalf_s{s}")
                nc.vector.reduce_sum(out=s_half, in_=s_cols,
                                     axis=mybir.AxisListType.X)
                fps = sps_pool.tile([128, 1], f32, tag="sb")
                nc.tensor.matmul(fps, lhsT=fold, rhs=s_half,
                                 start=True, stop=True)
                invs = sp.tile([128, 1], f32, tag="invs", name=f"invs_s{s}")
                nc.vector.reciprocal(out=invs, in_=fps)
                blk = sp.tile([128, 128], f32r, tag="blk", name=f"blk_s{s}")
                nc.vector.tensor_scalar_mul(blk, in0=mask, scalar1=invs)
                wsc = []
                for h in range(2):
                    wt = sp.tile([128, 128], f32r, tag=f"wsc{h}",
                                 name=f"wsc{h}_s{s}")
                    nc.vector.tensor_scalar_mul(wt, in0=ws[h], scalar1=invs)
                    wsc.append(wt)

                # ---- phase 2 ----
                for j in range(NCHUNK):
                    q, off = divmod(j * CHUNK, XT)
                    Ej = E[:, j * CHUNK:(j + 1) * CHUNK]
                    sps = sps_pool.tile([128, CHUNK], f32, tag="sb")
                    nc.tensor.matmul(sps, lhsT=blk, rhs=Ej,
                                     start=True, stop=True)
                    # reciprocal in place in PSUM. The reference's +1e-9
                    # is dropped: data-measured min column-sum is 7e-6, so
                    # the eps shifts the result by <1.5e-4 of the (small)
                    # attention branch -- far below fp32-gate noise.
                    nc.vector.reciprocal_approx_fast(out=sps, in_=sps)
                    nc.vector.tensor_mul(Ej, Ej.bitcast(f32), sps)
                    if True:
                        for h in range(2):
                            # z_big [128, 2, CHUNK]: both n-halves
                            zb = zps_pool.tile([128, 2, CHUNK], f32, tag="z")
                            for nh in range(2):
                                nc.tensor.matmul(
                                    zb[:, nh, :],
                                    lhsT=wsc[h][64 * nh:64 * nh + 64, :],
                                    rhs=Ej[64 * nh:64 * nh + 64, :],
                                    start=True, stop=True)
                            nc.vector.tensor_add(
                                zb, zb, xt[h, q][:, :, off:off + CHUNK])
                            st = stp.tile([128, 2, CHUNK], f32, tag="st")
                            nc.scalar.activation(
                                out=st, in_=zb,
                                func=mybir.ActivationFunctionType.Relu,
                                bias=bnb[:, h:h + 1], scale=1.0)
                            nc.sync.dma_start(
                                out=(y_d[s, 128 * h:128 * h + 128, :]
                                     .rearrange("p (a b) -> p a b", a=2)
                                     [:, :, j * CHUNK:(j + 1) * CHUNK]),
                                in_=st)
    nc.compile()
    return nc


def _consts(conv1_w, conv1_b, mk_w, mv_w, conv2_w, bn_gamma, bn_beta,
            bn_mean, bn_var):
    c1 = np.asarray(conv1_w, dtype=np.float64)
    mk = np.asarray(mk_w, dtype=np.float64)
    mv = np.asarray(mv_w, dtype=np.float64)
    c2 = np.asarray(conv2_w, dtype=np.float64)
    g = np.asarray(bn_gamma, dtype=np.float64)
    be = np.asarray(bn_beta, dtype=np.float64)
    mu = np.asarray(bn_mean, dtype=np.float64)
    va = np.asarray(bn_var, dtype=np.float64)

    A = mk @ c1                                    # [64, 256]
    inv = g / np.sqrt(va + BN_EPS)
    Bm = inv[:, None] * (c2 @ mv)                  # [256, 64]
    bias = be - mu * inv                           # [256]

    AT = np.ascontiguousarray(A.T, dtype=np.float32)      # [256, 64]
    a1t = AT[:128]
    a2t = AT[128:]
    wt = []
    for h in range(2):
        bh = np.ascontiguousarray(Bm[128 * h:128 * h + 128].T,
                                  dtype=np.float32)       # [64, 128]
        wt.append(np.concatenate([bh, bh], axis=0))       # [128, 128]
    k = np.arange(128)
    fold = (k[:, None] % 64 == k[None, :] % 64).astype(np.float32)
    mask = (k[:, None] // 64 == k[None, :] // 64).astype(np.float32)
    bnb = np.stack([bias[:128], bias[128:]], axis=1).astype(np.float32)
    ident = np.eye(128, dtype=np.float32)
    return {"a1t": a1t, "a2t": a2t, "w0": wt[0], "w1": wt[1],
            "fold": fold, "mask": mask, "bnb": bnb, "ident": ident}


def kernel(x, conv1_w, conv1_b, mk_w, mv_w, conv2_w, bn_gamma, bn_beta,
           bn_mean, bn_var):
    x = np.asarray(x, dtype=np.float32)
    consts = _consts(conv1_w, conv1_b, mk_w, mv_w, conv2_w, bn_gamma,
                     bn_beta, bn_mean, bn_var)
    if "nc" not in _cache:
        _cache["nc"] = _build()
    nc = _cache["nc"]

    xr = x.reshape(NCORES, SPC, C, N)
    in_maps = [dict(consts, xin=np.ascontiguousarray(xr[c]))
               for c in range(NCORES)]
    trace = bool(int(os.environ.get("KERNEL_TRACE", "0")))
    res = run_bass_kernel_spmd(nc, in_maps, list(range(NCORES)), trace=trace)
    _cache["exec_time_ns"] = res.exec_time_ns
    _cache["trace"] = res.instructions_and_trace
    out = np.stack([res.results[c]["y"] for c in range(NCORES)])
    return out.reshape(B_FULL, C, H, W)
